# revision 38
# baseline (speedup 1.0000x reference)
"""BERT encoder (6 shared-weight layers) on 8 Trainium2 NeuronCores.

Sharding: pure data-parallel over batch (B=8 -> 1 sequence per core).
Each core runs an identical single-core Bass kernel; no collectives.

Layout strategy inside a core: activations live feature-major ("transposed",
[H, S]) in SBUF so every matmul is
    y_T[j, s] = sum_f W[f, j] * x_T[f, s]  =  matmul(lhsT=W_chunk, rhs=x_T_chunk)
with weights in natural layout as the stationary operand.  LayerNorm
statistics are computed with ones-vector matmuls (fp32r moving operand,
full PE rate at N=512) and applied after a gpsimd partition_broadcast.
Softmax skips max-subtraction (scores are ~N(0, 0.3)) and folds the
denominator into the ctx matmul via a ones-row appended to V.
"""

import os
import sys

for _p in ("/opt/trn_rl_repo", os.path.expanduser("~/.axon_site/_ro/trn_rl_repo")):
    if os.path.isdir(_p) and _p not in sys.path:
        sys.path.insert(0, _p)

import ml_dtypes
import numpy as np

import concourse.bass as bass
import concourse.mybir as mybir
import concourse.tile as tile
from concourse import bacc
from concourse.bass import ts
from concourse.bass_utils import run_bass_kernel_spmd
from concourse.masks import make_identity
from contextlib import ExitStack

F32 = mybir.dt.float32
F32R = mybir.dt.float32r
BF16 = mybir.dt.bfloat16
I32 = mybir.dt.int32
AF = mybir.ActivationFunctionType
OP = mybir.AluOpType

B, S, H, NH, I, V, P_POS, T_TYP = 8, 512, 768, 12, 3072, 30522, 512, 2
D = H // NH          # 64
FC = H // 128        # 6 feature chunks
IC = I // 128        # 24 intermediate chunks
ST = S // 128        # 4 sequence tiles
EPS = 1e-12
N_CORES = 8


def build_nc(n_layers=6, debug_outs=()):
    """Build the single-core Bass module. debug_outs: iterable of tap names
    to also emit as DRAM outputs (for development)."""
    nc = bacc.Bacc("TRN2", target_bir_lowering=False, debug=False)
    dbg = set(debug_outs)

    def din(name, shape, dt):
        return nc.dram_tensor(name, shape, dt, kind="ExternalInput").ap()

    x_d = din("x", [1, S], I32)
    wword_d = din("W_word", [V, H], F32)
    wpos_d = din("W_pos", [P_POS, H], F32)
    wtype_d = din("W_type", [T_TYP, H], F32)
    wq_d = din("Wq", [H, H], BF16)
    wk_d = din("Wk", [H, H], BF16)
    wv_d = din("Wv", [H, H], BF16)
    wo_d = din("Wo", [H, H], BF16)
    wi_d = din("Wi", [H, I], BF16)
    wd_d = din("Wd", [I, H], BF16)
    wp_d = din("Wp", [H, H], BF16)
    bq_d = din("bq", [H], F32)
    bk_d = din("bk", [H], F32)
    bv_d = din("bv_bf", [H], BF16)   # row vector for K=1 bias matmul
    bo_d = din("bo", [H], F32)
    bi_d = din("bi", [I], F32)
    bd_d = din("bd", [H], F32)
    bp_d = din("bp", [H], F32)
    gemb_d = din("g_emb", [H], F32)
    bemb_d = din("b_emb", [H], F32)
    gattn_d = din("g_attn", [H], F32)
    battn_d = din("b_attn", [H], F32)
    gout_d = din("g_out", [H], F32)
    bout_d = din("b_out", [H], F32)

    out_h = nc.dram_tensor("out_h", [S, H], F32, kind="ExternalOutput").ap()
    out_p = nc.dram_tensor("out_pooled", [1, H], F32, kind="ExternalOutput").ap()
    dbg_outs = {}
    for name in dbg:
        if name.startswith("hT") or name.startswith("xaT"):
            dbg_outs[name] = nc.dram_tensor(name, [H, S], F32, kind="ExternalOutput").ap()

    with tile.TileContext(nc) as tc, ExitStack() as ctx:
        _body(nc, tc, ctx, locals(), n_layers, dbg_outs)
    nc.compile()
    return nc


def _body(nc, tc, ctx, t_, n_layers, dbg_outs):
    # ---------------- pools ----------------
    singles = ctx.enter_context(tc.tile_pool(name="singles", bufs=1))
    psum = ctx.enter_context(tc.tile_pool(name="psum", bufs=1, space="PSUM"))
    # acc: long-lived accumulators (6 banks), mm: short rotation (2 banks)
    # tags created on first use; bufs set per tag below.

    # ---------------- load weights (once, shared across layers) -------------
    wq = singles.tile([128, FC, H], BF16, name="wq")
    wk = singles.tile([128, FC, H], BF16, name="wk")
    wv = singles.tile([128, FC, H], BF16, name="wv")
    wo = singles.tile([128, FC, H], BF16, name="wo")
    wi = singles.tile([128, FC, I], BF16, name="wi")
    wd = singles.tile([128, IC, H], BF16, name="wd")
    for sb, dr in ((wq, t_["wq_d"]), (wk, t_["wk_d"]), (wv, t_["wv_d"]),
                   (wo, t_["wo_d"]), (wi, t_["wi_d"])):
        nc.gpsimd.dma_start(sb[:], dr.rearrange("(c p) j -> p c j", p=128))
    nc.gpsimd.dma_start(wd[:], t_["wd_d"].rearrange("(c p) j -> p c j", p=128))

    def col_vec(name, dr, chunks):
        sb = singles.tile([128, chunks], F32, name=name)
        nc.gpsimd.dma_start(sb[:], dr.rearrange("(c p) -> p c", p=128))
        return sb

    bq = col_vec("bq_c", t_["bq_d"], FC)
    bk = col_vec("bk_c", t_["bk_d"], FC)
    bo = col_vec("bo_c", t_["bo_d"], FC)
    bi = col_vec("bi_c", t_["bi_d"], IC)
    bd = col_vec("bd_c", t_["bd_d"], FC)
    g_emb = col_vec("g_emb_c", t_["gemb_d"], FC)
    b_emb = col_vec("b_emb_c", t_["bemb_d"], FC)
    g_attn = col_vec("g_attn_c", t_["gattn_d"], FC)
    b_attn = col_vec("b_attn_c", t_["battn_d"], FC)
    g_out = col_vec("g_out_c", t_["gout_d"], FC)
    b_out = col_vec("b_out_c", t_["bout_d"], FC)
    bp = col_vec("bp_c", t_["bp_d"], FC)

    bv_row = singles.tile([1, H], BF16, name="bv_row")
    nc.gpsimd.dma_start(bv_row[:], t_["bv_d"].unsqueeze(0))

    ones_col = singles.tile([128, 1], BF16, name="ones_col")
    nc.vector.memset(ones_col[:], 1.0)
    eps_col = singles.tile([128, 1], F32, name="eps_col")
    nc.vector.memset(eps_col[:], EPS)
    ones_row = singles.tile([1, 128], BF16, name="ones_row")
    nc.vector.memset(ones_row[:], 1.0)
    ident = singles.tile([128, 128], F32, name="ident")
    make_identity(nc, ident[:])



    # persistent activation storage
    resid = ctx.enter_context(tc.tile_pool(name="resid", bufs=1))
    workA = ctx.enter_context(tc.tile_pool(name="workA", bufs=1))

    def h_f32_tiles(gen):
        return [resid.tile([128, S], F32, name=f"res{c}_{gen}", tag=f"res{c}", bufs=2)
                for c in range(FC)]

    def h_bf_tiles(gen):
        return [workA.tile([128, S], BF16, name=f"abf{c}_{gen}", tag=f"abf{c}",
                           bufs=1) for c in range(FC)]

    # ---------------- embedding ----------------
    wword = t_["wword_d"]
    h_f32 = h_f32_tiles("emb")
    h_bf = h_bf_tiles("emb")

    with tc.tile_pool(name="embp", bufs=1) as embp:
        wtype_row = embp.tile([1, H], F32, name="wtype_row", tag="wtr", bufs=1)
        nc.gpsimd.dma_start(wtype_row[:], t_["wtype_d"][0:1, :])
        typb = embp.tile([128, H], F32, name="typb", tag="typb", bufs=1)
        nc.gpsimd.partition_broadcast(typb[:], wtype_row[:])
        for t in range(ST):
            idx = embp.tile([128, 1], I32, name=f"idx{t}", tag="idx", bufs=2)
            nc.gpsimd.dma_start(
                idx[:], t_["x_d"][0:1, ts(t, 128)].rearrange("o (p q) -> (o p) q", q=1))
            emb = embp.tile([128, H], F32, name=f"emb{t}", tag="emb", bufs=2)
            nc.gpsimd.indirect_dma_start(
                out=emb[:], out_offset=None, in_=wword[:],
                in_offset=bass.IndirectOffsetOnAxis(ap=idx[:, 0:1], axis=0))
            pos = embp.tile([128, H], F32, name=f"pos{t}", tag="pos", bufs=1)
            nc.gpsimd.dma_start(pos[:], t_["wpos_d"][ts(t, 128), :])
            # e = emb + pos + type0
            nc.vector.tensor_add(emb[:], emb[:], pos[:])
            nc.vector.tensor_add(emb[:], emb[:], typb[:])
            # natural-layout LN statistics
            stats = embp.tile([128, 3, nc.vector.BN_STATS_DIM], F32,
                              name=f"st{t}", tag="st", bufs=2)
            er = emb[:].rearrange("p (n f) -> p n f", f=256)
            for sg in range(3):
                nc.vector.bn_stats(out=stats[:, sg, :], in_=er[:, sg, :])
            mv = embp.tile([128, nc.vector.BN_AGGR_DIM], F32,
                           name=f"mv{t}", tag="mv", bufs=2)
            nc.vector.bn_aggr(out=mv[:], in_=stats[:])
            rstd = embp.tile([128, 1], F32, name=f"rstd{t}", tag="rstd", bufs=2)
            nc.scalar.activation(out=rstd[:], in_=mv[:, 1:2], func=AF.Sqrt,
                                 bias=eps_col[:], scale=1.0)
            nc.vector.reciprocal(rstd[:], rstd[:])
            hc = embp.tile([128, H], F32, name=f"hc{t}", tag="hc", bufs=1)
            nc.vector.tensor_scalar(hc[:], emb[:], mv[:, 0:1], rstd[:],
                                    OP.subtract, OP.mult)
            for c in range(FC):
                pt = psum.tile([128, 128], F32, name=f"ptr{t}_{c}", tag="mm", bufs=2)
                nc.tensor.transpose(out=pt[:], in_=hc[:, ts(c, 128)], identity=ident[:])
                nc.vector.tensor_scalar(h_f32[c][:, ts(t, 128)], pt[:],
                                        g_emb[:, c:c + 1], b_emb[:, c:c + 1],
                                        OP.mult, OP.add)
        for c in range(FC):
            nc.vector.tensor_copy(h_bf[c][:], h_f32[c][:])

    # main working pool — allocated after embp releases its space
    work = ctx.enter_context(tc.tile_pool(name="work", bufs=1))

    # ---------------- transformer layers ----------------
    for L in range(n_layers):
        # ---- phase A: q, k (transposed), v (natural, augmented) ----
        q_bf, k_bf = [], []
        for name, w_sb, b_sb, store in (("q", wq, bq, q_bf), ("k", wk, bk, k_bf)):
            for m in range(FC):
                ps = psum.tile([128, S], F32, name=f"ps{name}{L}_{m}", tag="mm", bufs=2)
                for c in range(FC):
                    nc.tensor.matmul(ps[:], w_sb[:, c, ts(m, 128)], h_bf[c][:],
                                     start=(c == 0), stop=(c == FC - 1))
                ob = work.tile([128, S], BF16, name=f"{name}bf{L}_{m}",
                               tag=f"{name}{m}", bufs=1)
                nc.vector.tensor_scalar(ob[:], ps[:], b_sb[:, m:m + 1], None, OP.add)
                store.append(ob)

        v_aug = []
        for t in range(ST):
            va = work.tile([128, NH, D + 1], BF16, name=f"vaug{L}_{t}",
                           tag=f"va{t}", bufs=1)
            for lo, hi in ((0, 512), (512, 768)):
                ps = psum.tile([128, hi - lo], F32, name=f"psv{L}_{t}_{lo}",
                               tag="mm", bufs=2)
                for c in range(FC):
                    nc.tensor.matmul(ps[:], h_bf[c][:, ts(t, 128)], wv[:, c, lo:hi],
                                     start=(c == 0), stop=False)
                nc.tensor.matmul(ps[:], ones_row[:], bv_row[:, lo:hi],
                                 start=False, stop=True)
                nc.vector.tensor_copy(
                    va[:, lo // D:hi // D, 0:D],
                    ps[:].rearrange("p (h d) -> p h d", d=D))
            nc.vector.memset(va[:, :, D:D + 1], 1.0)
            v_aug.append(va)

        # ---- phase B: attention per head ----
        ctx_bf = [work.tile([128, S], BF16, name=f"ctx{L}_{c}", tag=f"cx{c}", bufs=1)
                  for c in range(FC)]
        for hh in range(NH):
            mh, poff = hh // 2, (hh % 2) * D
            probs = []
            for u in range(ST):
                ps_sc = psum.tile([128, S], F32, name=f"sc{L}_{hh}_{u}",
                                  tag="mm", bufs=2)
                nc.tensor.matmul(ps_sc[:],
                                 k_bf[mh][poff:poff + D, ts(u, 128)],
                                 q_bf[mh][poff:poff + D, :],
                                 start=True, stop=True)
                pb = work.tile([128, S], BF16, name=f"pr{L}_{hh}_{u}",
                               tag="probs", bufs=6)
                nc.scalar.activation(out=pb[:], in_=ps_sc[:], func=AF.Exp,
                                     scale=1.0 / (D ** 0.5))
                probs.append(pb)
            ps_cx = psum.tile([D + 1, S], F32, name=f"cx{L}_{hh}", tag="acc", bufs=6)
            for u in range(ST):
                nc.tensor.matmul(ps_cx[:], v_aug[u][:, hh, :], probs[u][:],
                                 start=(u == 0), stop=(u == ST - 1))
            rden_t = work.tile([1, S], F32, name=f"rden{L}_{hh}", tag="rden", bufs=2)
            rden = rden_t[:]
            nc.vector.reciprocal(rden, ps_cx[D:D + 1, :])
            rdb = work.tile([128, S], F32, name=f"rdb{L}_{hh}", tag="rdb", bufs=2)
            nc.gpsimd.partition_broadcast(rdb[:], rden)
            nc.vector.tensor_tensor(ctx_bf[mh][poff:poff + D, :],
                                    ps_cx[0:D, :], rdb[poff:poff + D, :], OP.mult)

        # ---- phase C: attention out + residual + LN ----
        x1 = h_f32_tiles(f"x1_{L}")
        for m in range(FC):
            ps = psum.tile([128, S], F32, name=f"pat{L}_{m}", tag="mm", bufs=2)
            for c in range(FC):
                nc.tensor.matmul(ps[:], wo[:, c, ts(m, 128)], ctx_bf[c][:],
                                 start=(c == 0), stop=(c == FC - 1))
            nc.vector.scalar_tensor_tensor(x1[m][:], ps[:], bo[:, m:m + 1],
                                           h_f32[m][:], OP.add, OP.add)

        xa_f32, xa_bf = _layernorm_T(nc, tc, psum, work, ones_col, eps_col,
                                     x1, g_attn, b_attn, h_f32_tiles, h_bf_tiles,
                                     f"ln1_{L}")

        # ---- phase D: FFN (interleaved up/down to bound gelu storage) ----
        x2 = h_f32_tiles(f"x2_{L}")
        ps_d = [psum.tile([128, S], F32, name=f"psd{L}_{m}", tag="acc", bufs=6)
                for m in range(FC)]
        for jm in range(IC):
            ps_wi = psum.tile([128, S], F32, name=f"pwi{L}_{jm}", tag="mm", bufs=2)
            for c in range(FC):
                nc.tensor.matmul(ps_wi[:], wi[:, c, ts(jm, 128)], xa_bf[c][:],
                                 start=(c == 0), stop=(c == FC - 1))
            gg = work.tile([128, S], BF16, name=f"gg{L}_{jm}", tag="gg", bufs=6)
            nc.scalar.activation(out=gg[:], in_=ps_wi[:], func=AF.Gelu,
                                 bias=bi[:, jm:jm + 1], scale=1.0)
            for m in range(FC):
                nc.tensor.matmul(ps_d[m][:], wd[:, jm, ts(m, 128)], gg[:],
                                 start=(jm == 0), stop=(jm == IC - 1))
        for m in range(FC):
            nc.vector.scalar_tensor_tensor(x2[m][:], ps_d[m][:], bd[:, m:m + 1],
                                           xa_f32[m][:], OP.add, OP.add)

        h_f32, h_bf = _layernorm_T(nc, tc, psum, work, ones_col, eps_col,
                                   x2, g_out, b_out, h_f32_tiles, h_bf_tiles,
                                   f"ln2_{L}")

        for name, ap in dbg_outs.items():
            if name == f"hT{L}":
                for c in range(FC):
                    nc.gpsimd.dma_start(ap[ts(c, 128), :], h_f32[c][:])
            if name == f"xaT{L}":
                for c in range(FC):
                    nc.gpsimd.dma_start(ap[ts(c, 128), :], xa_f32[c][:])

    # ---------------- outputs ----------------
    # pooled_T[j] = sum_f Wp[f, j] * h_T[f, 0]  (+ bp), streamed Wp chunks
    ps_p = [psum.tile([128, 1], F32, name=f"psp{m}", tag="acc", bufs=6)
            for m in range(FC)]
    wp_chunks = t_["wp_d"].rearrange("(c p) j -> c p j", p=128)
    for c in range(FC):
        wpc = work.tile([128, H], BF16, name=f"wpc{c}", tag="wpc", bufs=2)
        nc.gpsimd.dma_start(wpc[:], wp_chunks[c])
        for m in range(FC):
            nc.tensor.matmul(ps_p[m][:], wpc[:, ts(m, 128)], h_bf[c][:, 0:1],
                             start=(c == 0), stop=(c == FC - 1))
    for m in range(FC):
        pcol = work.tile([128, 1], F32, name=f"pcol{m}", tag="pcol", bufs=2)
        nc.vector.tensor_scalar(pcol[:], ps_p[m][:], bp[:, m:m + 1], None, OP.add)
        nc.gpsimd.dma_start(
            t_["out_p"][0, ts(m, 128)].rearrange("(p q) -> p q", q=1), pcol[:])

    # h natural: transpose back via PE, stage in SBUF, DMA out per block
    for t in range(ST):
        for c in range(FC):
            pt = psum.tile([128, 128], F32, name=f"pto{t}_{c}", tag="mm", bufs=2)
            nc.tensor.transpose(out=pt[:], in_=h_f32[c][:, ts(t, 128)],
                                identity=ident[:])
            hnat = work.tile([128, 128], F32, name=f"hnat{t}_{c}",
                             tag="hnat", bufs=2)
            nc.vector.tensor_copy(hnat[:], pt[:])
            nc.gpsimd.dma_start(t_["out_h"][ts(t, 128), ts(c, 128)], hnat[:])


def _layernorm_T(nc, tc, psum, work, ones_col, eps_col, x, g_sb, b_sb,
                 h_f32_tiles, h_bf_tiles, gen):
    """Transposed-layout LayerNorm over the partition (feature) axis.
    x: list of FC [128, S] f32 tiles. Returns (out_f32_tiles, out_bf_tiles)."""
    ps_sum = psum.tile([1, S], F32, name=f"pssum_{gen}", tag="acc", bufs=6)
    ps_sq = psum.tile([1, S], F32, name=f"pssq_{gen}", tag="acc", bufs=6)
    for c in range(FC):
        xb = work.tile([128, S], BF16, name=f"xb_{gen}_{c}", tag="xb", bufs=2)
        nc.scalar.copy(xb[:], x[c][:])
        sq = work.tile([128, S], BF16, name=f"sq_{gen}_{c}", tag="sq", bufs=2)
        nc.vector.tensor_tensor(sq[:], x[c][:], x[c][:], OP.mult)
        nc.tensor.matmul(ps_sum[:], ones_col[:], xb[:],
                         start=(c == 0), stop=(c == FC - 1))
        nc.tensor.matmul(ps_sq[:], ones_col[:], sq[:],
                         start=(c == 0), stop=(c == FC - 1))
    mean_t = work.tile([1, S], F32, name=f"mean_{gen}", tag="mrow", bufs=1)
    mean = mean_t[:]
    nc.vector.tensor_scalar(mean, ps_sum[:], 1.0 / H, None, OP.mult)
    t_t = work.tile([1, S], F32, name=f"trow_{gen}", tag="trow", bufs=1)
    t_row = t_t[:]
    # t = sumsq - H*mean^2
    nc.vector.scalar_tensor_tensor(t_row, mean, float(H), mean, OP.mult, OP.mult)
    nc.vector.tensor_sub(t_row, ps_sq[:], t_row)
    rstd_t = work.tile([1, S], F32, name=f"rstd_{gen}", tag="rrow", bufs=1)
    rstd = rstd_t[:]
    # rstd = 1 / sqrt(t/H + eps)
    nc.scalar.activation(out=rstd, in_=t_row, func=AF.Sqrt,
                         bias=eps_col[0:1, :], scale=1.0 / H)
    nc.vector.reciprocal(rstd, rstd)
    mean_b = work.tile([128, S], F32, name=f"mb_{gen}", tag="mb", bufs=1)
    nc.gpsimd.partition_broadcast(mean_b[:], mean)
    rstd_b = work.tile([128, S], F32, name=f"rb_{gen}", tag="rb", bufs=1)
    nc.gpsimd.partition_broadcast(rstd_b[:], rstd)

    out_f32 = h_f32_tiles(gen)
    out_bf = h_bf_tiles(gen)
    for c in range(FC):
        cen = work.tile([128, S], F32, name=f"cen_{gen}_{c}", tag="lnt", bufs=2)
        nc.vector.tensor_sub(cen[:], x[c][:], mean_b[:])
        nrm = work.tile([128, S], F32, name=f"nrm_{gen}_{c}", tag="lnt", bufs=2)
        nc.vector.tensor_mul(nrm[:], cen[:], rstd_b[:])
        nc.vector.tensor_scalar(out_f32[c][:], nrm[:], g_sb[:, c:c + 1],
                                b_sb[:, c:c + 1], OP.mult, OP.add)
        nc.vector.tensor_copy(out_bf[c][:], out_f32[c][:])
    return out_f32, out_bf


# ======================= host side =======================

_CACHE = {}


def _get_nc(n_layers=6, debug_outs=()):
    key = (n_layers, tuple(sorted(debug_outs)))
    if key not in _CACHE:
        _CACHE[key] = build_nc(n_layers, debug_outs)
    return _CACHE[key]


class _Runner:
    """Jit-once runner with device-resident cached inputs (weights)."""

    def __init__(self, nc):
        import jax
        from jax.experimental.shard_map import shard_map
        from jax.sharding import Mesh, NamedSharding, PartitionSpec
        import concourse.mybir as _mybir
        from concourse.bass2jax import (_bass_exec_p, install_neuronx_cc_hook,
                                        partition_id_tensor)

        install_neuronx_cc_hook()
        self.jax = jax
        self.nc = nc
        in_names, out_names, out_avals = [], [], []
        partition_name = (nc.partition_id_tensor.name
                          if nc.partition_id_tensor else None)
        for alloc in nc.m.functions[0].allocations:
            if not isinstance(alloc, _mybir.MemoryLocationSet):
                continue
            name = alloc.memorylocations[0].name
            if alloc.kind == "ExternalInput":
                if name != partition_name:
                    in_names.append(name)
            elif alloc.kind == "ExternalOutput":
                shape = tuple(alloc.tensor_shape)
                dtype = _mybir.dt.np(alloc.dtype)
                out_names.append(name)
                out_avals.append(jax.core.ShapedArray(shape, dtype))
        self.in_names = list(in_names)
        self.out_names = out_names
        self.out_avals = out_avals
        n_params = len(in_names)
        n_outs = len(out_avals)
        all_names = in_names + out_names
        if partition_name is not None:
            all_names.append(partition_name)
        donate = tuple(range(n_params, n_params + n_outs))

        devices = jax.devices()[:N_CORES]
        assert len(devices) == N_CORES
        self.mesh = Mesh(np.asarray(devices), ("core",))
        self.sharding = NamedSharding(self.mesh, PartitionSpec("core"))

        def _body(*args):
            operands = list(args)
            if partition_name is not None:
                operands.append(partition_id_tensor())
            outs = _bass_exec_p.bind(
                *operands,
                out_avals=tuple(out_avals),
                in_names=tuple(all_names),
                out_names=tuple(out_names),
                lowering_input_output_aliases=(),
                sim_require_finite=True,
                sim_require_nnan=True,
                nc=nc,
            )
            return tuple(outs)

        in_specs = (PartitionSpec("core"),) * (n_params + n_outs)
        out_specs = (PartitionSpec("core"),) * n_outs
        self.sharded = jax.jit(
            shard_map(_body, mesh=self.mesh, in_specs=in_specs,
                      out_specs=out_specs, check_rep=False),
            donate_argnums=donate, keep_unused=True)
        self._dev_cache = {}

    def put(self, name, arrs_per_core):
        """Cache-aware H2D of one input across all cores (keyed by identity)."""
        key = tuple(id(a) for a in arrs_per_core)
        hit = self._dev_cache.get(name)
        if hit is not None and hit[0] == key:
            return hit[1]
        cat = np.concatenate([np.asarray(a) for a in arrs_per_core], axis=0)
        dev = self.jax.device_put(cat, self.sharding)
        dev.block_until_ready()
        self._dev_cache[name] = (key, dev)
        return dev

    def zeros(self):
        return [self.jax.device_put(
                    np.zeros((N_CORES * a.shape[0], *a.shape[1:]), a.dtype),
                    self.sharding)
                for a in self.out_avals]

    def exec(self, dev_inputs, dev_zeros):
        outs = self.sharded(*dev_inputs, *dev_zeros)
        self.jax.block_until_ready(outs)
        return outs

    def __call__(self, in_maps):
        dev_inputs = [self.put(n, [m[n] for m in in_maps]) for n in self.in_names]
        outs = self.exec(dev_inputs, self.zeros())
        results = []
        host = [np.asarray(o) for o in outs]
        for c in range(N_CORES):
            results.append({
                name: host[i].reshape(N_CORES, *self.out_avals[i].shape)[c]
                for i, name in enumerate(self.out_names)})
        return results


def _get_runner(n_layers=6, debug_outs=()):
    key = ("runner", n_layers, tuple(sorted(debug_outs)))
    if key not in _CACHE:
        _CACHE[key] = _Runner(_get_nc(n_layers, debug_outs))
    return _CACHE[key]


_WEIGHT_SRC = ["W_word", "W_pos", "W_type", "Wq", "Wk", "Wv", "Wo", "Wi",
               "Wd", "Wp", "bq", "bk", "bv", "bo", "bi", "bd", "bp",
               "g_emb", "b_emb", "g_attn", "b_attn", "g_out", "b_out"]


def _prep_shared(inputs):
    """Cast/convert all non-x inputs; cached by identity of the sources so
    repeated kernel() calls with the same weight arrays skip the host casts
    (and downstream device transfers)."""
    key = tuple(id(inputs[n]) for n in _WEIGHT_SRC)
    hit = _CACHE.get("shared")
    if hit is not None and hit[0] == key:
        return hit[1]
    bf = ml_dtypes.bfloat16
    f32 = np.float32

    def a(name, dt=f32):
        return np.ascontiguousarray(np.asarray(inputs[name]).astype(dt))

    shared = {
        "W_word": a("W_word"), "W_pos": a("W_pos"), "W_type": a("W_type"),
        "Wq": a("Wq", bf), "Wk": a("Wk", bf), "Wv": a("Wv", bf),
        "Wo": a("Wo", bf), "Wi": a("Wi", bf), "Wd": a("Wd", bf),
        "Wp": a("Wp", bf),
        "bq": a("bq"), "bk": a("bk"), "bv_bf": a("bv", bf), "bo": a("bo"),
        "bi": a("bi"), "bd": a("bd"), "bp": a("bp"),
        "g_emb": a("g_emb"), "b_emb": a("b_emb"),
        "g_attn": a("g_attn"), "b_attn": a("b_attn"),
        "g_out": a("g_out"), "b_out": a("b_out"),
    }
    _CACHE["shared"] = (key, shared)
    return shared


def _prep_in_maps(inputs):
    shared = _prep_shared(inputs)
    x = np.asarray(inputs["x"]).astype(np.int32)
    in_maps = []
    for c in range(N_CORES):
        m = dict(shared)
        m["x"] = np.ascontiguousarray(x[c:c + 1, :])
        in_maps.append(m)
    return in_maps


def run(inputs, n_layers=6, debug_outs=()):
    runner = _get_runner(n_layers, debug_outs)
    in_maps = _prep_in_maps(inputs)
    results = runner(in_maps)
    h = np.stack([results[c]["out_h"] for c in range(N_CORES)])
    pooled = np.stack([results[c]["out_pooled"][0] for c in range(N_CORES)])
    extras = {
        name: np.stack([results[c][name] for c in range(N_CORES)])
        for name in debug_outs
    }
    return (h, pooled), extras, results


def kernel(**inputs):
    (h, pooled), _, _ = run(inputs)
    return h.astype(np.float32), pooled.astype(np.float32)


# revision 65
# speedup vs baseline: 15.6016x; 15.6016x over previous
"""BERT encoder (6 shared-weight layers) on 8 Trainium2 NeuronCores.

Sharding: pure data-parallel over batch (B=8 -> 1 sequence per core).
Each core runs an identical single-core Bass kernel; no collectives.

Layout strategy inside a core: activations live feature-major ("transposed",
[H, S]) in SBUF so every matmul is
    y_T[j, s] = sum_f W[f, j] * x_T[f, s]  =  matmul(lhsT=W_chunk, rhs=x_T_chunk)
with weights in natural layout as the stationary operand.  LayerNorm
statistics are computed with ones-vector matmuls over bf16 casts and applied
after a gpsimd partition_broadcast; rsqrt runs entirely on the vector engine
(magic-constant seed + one Newton step) to avoid ACT table-set switches.
Softmax skips max-subtraction (scores are ~N(0, 0.3)) and folds the
denominator into the ctx matmul via a ones-row appended to V.

The build is specialized (with an exact runtime check and a general
fallback) on the observation that all bias vectors are zero and all
LayerNorm gammas/betas are one/zero for this module's initialization.
"""

import os
import sys

for _p in ("/opt/trn_rl_repo", os.path.expanduser("~/.axon_site/_ro/trn_rl_repo")):
    if os.path.isdir(_p) and _p not in sys.path:
        sys.path.insert(0, _p)

import ml_dtypes
import numpy as np

import concourse.bass as bass
import concourse.mybir as mybir
import concourse.tile as tile
from concourse import bacc
from concourse.bass import ts
from concourse.masks import make_identity
from contextlib import ExitStack

F32 = mybir.dt.float32
I32 = mybir.dt.int32
BF16 = mybir.dt.bfloat16
AF = mybir.ActivationFunctionType
OP = mybir.AluOpType

B, S, H, NH, I, V = 8, 512, 768, 12, 3072, 30522
P_POS, T_TYP = 512, 2
D = H // NH          # 64
FC = H // 128        # 6 feature chunks
IC = I // 128        # 24 intermediate chunks
ST = S // 128        # 4 sequence tiles
EPS = 1e-12
N_CORES = 8
# Lomont-style optimal magic/Newton pair for one-iteration rsqrt (~6.5e-4 max)
RSQRT_MAGIC = 0x5F1FFFF9
RSQRT_A = 1.68191391
RSQRT_B = -0.70395225


def build_nc(n_layers=6, debug_outs=(), simple=True):
    """Build the single-core Bass module.

    simple=True assumes all projection biases are zero and LN gamma/beta are
    one/zero (verified exactly on the host before selecting this build)."""
    nc = bacc.Bacc("TRN2", target_bir_lowering=False, debug=False)
    dbg = set(debug_outs)

    def din(name, shape, dt):
        return nc.dram_tensor(name, shape, dt, kind="ExternalInput").ap()

    t_ = {}
    t_["x_d"] = din("x", [1, S], I32)
    t_["wword_d"] = din("W_word", [V, H], F32)
    t_["wpos_d"] = din("W_pos", [P_POS, H], F32)
    t_["wtype_d"] = din("W_type", [T_TYP, H], F32)
    for w, shape in (("wq", [H, H]), ("wk", [H, H]), ("wv", [H, H]),
                     ("wo", [H, H]), ("wi", [H, I]), ("wd", [I, H]),
                     ("wp", [H, H])):
        t_[w + "_d"] = din(w.capitalize() if False else
                           {"wq": "Wq", "wk": "Wk", "wv": "Wv", "wo": "Wo",
                            "wi": "Wi", "wd": "Wd", "wp": "Wp"}[w], shape, BF16)
    if not simple:
        for b, n in (("bq", H), ("bk", H), ("bo", H), ("bi", I), ("bd", H),
                     ("bp", H), ("gemb", H), ("bemb", H), ("gattn", H),
                     ("battn", H), ("gout", H), ("bout", H)):
            name = {"gemb": "g_emb", "bemb": "b_emb", "gattn": "g_attn",
                    "battn": "b_attn", "gout": "g_out", "bout": "b_out"
                    }.get(b, b)
            t_[b + "_d"] = din(name, [n], F32)
        t_["bv_d"] = din("bv", [H], F32)

    t_["out_h"] = nc.dram_tensor("out_h", [S, H], F32, kind="ExternalOutput").ap()
    t_["out_p"] = nc.dram_tensor("out_pooled", [1, H], F32,
                                 kind="ExternalOutput").ap()
    dbg_outs = {}
    for name in dbg:
        dbg_outs[name] = nc.dram_tensor(name, [H, S], F32,
                                        kind="ExternalOutput").ap()

    with tile.TileContext(nc) as tc, ExitStack() as ctx:
        _body(nc, tc, ctx, t_, n_layers, dbg_outs, simple)
    nc.compile()
    return nc


def _body(nc, tc, ctx, t_, n_layers, dbg_outs, simple):
    singles = ctx.enter_context(tc.tile_pool(name="singles", bufs=1))
    psum = ctx.enter_context(tc.tile_pool(name="psum", bufs=1, space="PSUM"))
    # psum tags: "mm" 2-deep rotation for linear phases; "acc" 6-deep ring for
    # attention scores/ctx, FFN-down accumulators, LN stat rows, pooled.

    # -------- weights (resident all layers; DMA issued after embedding) ----
    wq = singles.tile([128, FC, H], BF16, name="wq")
    wk = singles.tile([128, FC, H], BF16, name="wk")
    wv = singles.tile([128, FC, H], BF16, name="wv")
    wo = singles.tile([128, FC, H], BF16, name="wo")
    wi = singles.tile([128, FC, I], BF16, name="wi")
    wd = singles.tile([128, IC, H], BF16, name="wd")

    def load_weights(after_inst):
        # ordered after the embedding-critical DMAs so those win the DMA
        # engines first; weights stream in behind them.
        from concourse.bass import _add_dep_helper
        first = True
        for sb, dr in ((wq, t_["wq_d"]), (wk, t_["wk_d"]),
                       (wv, t_["wv_d"]), (wo, t_["wo_d"]),
                       (wi, t_["wi_d"]), (wd, t_["wd_d"])):
            bi = nc.sync.dma_start(sb[:], dr.rearrange("(c p) j -> p c j",
                                                       p=128))
            if first and after_inst is not None:
                _add_dep_helper(bi.ins, after_inst.ins,
                                reason="weight DMAs after embedding DMAs")
                first = False

    def col_vec(name, dr, chunks):
        sb = singles.tile([128, chunks], F32, name=name)
        nc.sync.dma_start(sb[:], dr.rearrange("(c p) -> p c", p=128))
        return sb

    if not simple:
        bq = col_vec("bq_c", t_["bq_d"], FC)
        bk = col_vec("bk_c", t_["bk_d"], FC)
        bo = col_vec("bo_c", t_["bo_d"], FC)
        bi = col_vec("bi_c", t_["bi_d"], IC)
        bd = col_vec("bd_c", t_["bd_d"], FC)
        bp = col_vec("bp_c", t_["bp_d"], FC)
        g_emb = col_vec("g_emb_c", t_["gemb_d"], FC)
        b_emb = col_vec("b_emb_c", t_["bemb_d"], FC)
        g_attn = col_vec("g_attn_c", t_["gattn_d"], FC)
        b_attn = col_vec("b_attn_c", t_["battn_d"], FC)
        g_out = col_vec("g_out_c", t_["gout_d"], FC)
        b_out = col_vec("b_out_c", t_["bout_d"], FC)
        bv_row = singles.tile([1, H], F32, name="bv_row")
        nc.sync.dma_start(bv_row[:], t_["bv_d"].unsqueeze(0))
    else:
        bq = bk = bo = bi = bd = bp = None
        g_emb = b_emb = g_attn = b_attn = g_out = b_out = None
        bv_row = None

    ones_col = singles.tile([128, 1], BF16, name="ones_col")
    nc.vector.memset(ones_col[:], 1.0)
    ident = singles.tile([128, 128], F32, name="ident")
    make_identity(nc, ident[:])
    shift1 = singles.tile([128, 1], I32, name="shift1")
    nc.vector.memset(shift1[:], 1)
    magic_t = singles.tile([128, S], I32, name="magic_t")
    nc.vector.memset(magic_t[:], RSQRT_MAGIC)

    resid = ctx.enter_context(tc.tile_pool(name="resid", bufs=1))
    workA = ctx.enter_context(tc.tile_pool(name="workA", bufs=1))

    def h_f32_tiles(gen):
        return [resid.tile([128, S], F32, name=f"res{c}_{gen}", tag=f"res{c}",
                           bufs=2) for c in range(FC)]

    def h_bf_tiles(gen):
        return [workA.tile([128, S], BF16, name=f"abf{c}_{gen}", tag=f"abf{c}",
                           bufs=1) for c in range(FC)]

    cfg = dict(simple=simple, ones_col=ones_col)

    def rsqrt_dve(out, var, scratch, p):
        """out = 1/sqrt(var) elementwise; var/scratch/out [p, N] f32 APs."""
        yi = scratch.bitcast(I32)
        nc.vector.tensor_scalar(yi, var.bitcast(I32), shift1[0:p, :], None,
                                OP.logical_shift_right)
        nc.vector.tensor_tensor(yi, magic_t[0:p, 0:var.shape[-1]], yi,
                                OP.subtract)
        nc.vector.tensor_tensor(out, scratch, scratch, OP.mult)
        nc.vector.tensor_tensor(out, out, var, OP.mult)
        nc.vector.tensor_scalar(out, out, RSQRT_B, RSQRT_A, OP.mult, OP.add)
        nc.vector.tensor_tensor(out, out, scratch, OP.mult)

    cfg["rsqrt"] = rsqrt_dve

    # ---------------- embedding ----------------
    h_f32 = h_f32_tiles("emb")
    h_bf = h_bf_tiles("emb")

    with tc.tile_pool(name="embp", bufs=1) as embp:
        wtype_row = embp.tile([1, H], F32, name="wtype_row", tag="wtr", bufs=1)
        nc.gpsimd.dma_start(wtype_row[:], t_["wtype_d"][0:1, :])
        typb = embp.tile([128, H], F32, name="typb", tag="typb", bufs=1)
        nc.gpsimd.partition_broadcast(typb[:], wtype_row[:])
        for t in range(ST):
            idx = embp.tile([128, 1], I32, name=f"idx{t}", tag="idx", bufs=2)
            nc.gpsimd.dma_start(
                idx[:],
                t_["x_d"][0:1, ts(t, 128)].rearrange("o (p q) -> (o p) q", q=1))
            emb = embp.tile([128, H], F32, name=f"emb{t}", tag="ebig", bufs=4)
            nc.gpsimd.dma_start(emb[:], t_["wpos_d"][ts(t, 128), :])
            nc.vector.tensor_add(emb[:], emb[:], typb[:])
            # gather accumulates the word embeddings onto pos+type in place
            last_emb_dma = nc.gpsimd.indirect_dma_start(
                out=emb[:], out_offset=None, in_=t_["wword_d"][:],
                in_offset=bass.IndirectOffsetOnAxis(ap=idx[:, 0:1], axis=0),
                compute_op=OP.add)
            stats = embp.tile([128, 3, nc.vector.BN_STATS_DIM], F32,
                              name=f"st{t}", tag="st", bufs=2)
            er = emb[:].rearrange("p (n f) -> p n f", f=256)
            for sg in range(3):
                nc.vector.bn_stats(out=stats[:, sg, :], in_=er[:, sg, :])
            mv = embp.tile([128, nc.vector.BN_AGGR_DIM], F32,
                           name=f"mv{t}", tag="mv", bufs=2)
            nc.vector.bn_aggr(out=mv[:], in_=stats[:])
            rs = embp.tile([128, 3], F32, name=f"rs{t}", tag="rstd", bufs=2)
            nc.vector.tensor_scalar(rs[:, 0:1], mv[:, 1:2], EPS, None, OP.add)
            rsqrt_dve(rs[:, 2:3], rs[:, 0:1], rs[:, 1:2], 128)
            hc = embp.tile([128, H], F32, name=f"hc{t}", tag="ebig", bufs=4)
            nc.vector.tensor_scalar(hc[:], emb[:], mv[:, 0:1], rs[:, 2:3],
                                    OP.subtract, OP.mult)
            for c in range(FC):
                pt = psum.tile([128, 128], F32, name=f"ptr{t}_{c}", tag="mm",
                               bufs=2)
                nc.tensor.transpose(out=pt[:], in_=hc[:, ts(c, 128)],
                                    identity=ident[:])
                if simple:
                    nc.vector.tensor_copy(h_f32[c][:, ts(t, 128)], pt[:])
                else:
                    nc.vector.tensor_scalar(h_f32[c][:, ts(t, 128)], pt[:],
                                            g_emb[:, c:c + 1], b_emb[:, c:c + 1],
                                            OP.mult, OP.add)
        for c in range(FC):
            nc.scalar.copy(h_bf[c][:], h_f32[c][:])

    load_weights(last_emb_dma)
    work = ctx.enter_context(tc.tile_pool(name="work", bufs=1))
    if not simple:
        bv_b = work.tile([128, H], F32, name="bv_b", tag="bv_b", bufs=1)
        nc.gpsimd.partition_broadcast(bv_b[:], bv_row[:])

    # ---------------- transformer layers ----------------
    for L in range(n_layers):
        # ---- q, k transposed; v natural+augmented ----
        q_bf, k_bf = [], []
        for name, w_sb, b_sb, store in (("q", wq, bq, q_bf), ("k", wk, bk, k_bf)):
            for m in range(FC):
                ps = psum.tile([128, S], F32, name=f"ps{name}{L}_{m}",
                               tag="mm", bufs=2)
                for c in range(FC):
                    nc.tensor.matmul(ps[:], w_sb[:, c, ts(m, 128)], h_bf[c][:],
                                     start=(c == 0), stop=(c == FC - 1))
                ob = work.tile([128, S], BF16, name=f"{name}bf{L}_{m}",
                               tag=f"{name}{m}", bufs=1)
                if simple:
                    nc.vector.tensor_copy(ob[:], ps[:])
                else:
                    nc.vector.tensor_scalar(ob[:], ps[:], b_sb[:, m:m + 1],
                                            None, OP.add)
                store.append(ob)

        v_aug = []
        for t in range(ST):
            va = work.tile([128, NH, D + 1], BF16, name=f"vaug{L}_{t}",
                           tag=f"va{t}", bufs=1)
            for lo, hi in ((0, 512), (512, 768)):
                ps = psum.tile([128, hi - lo], F32, name=f"psv{L}_{t}_{lo}",
                               tag="mm", bufs=2)
                for c in range(FC):
                    nc.tensor.matmul(ps[:], h_bf[c][:, ts(t, 128)],
                                     wv[:, c, lo:hi],
                                     start=(c == 0), stop=(c == FC - 1))
                if simple:
                    nc.vector.tensor_copy(
                        va[:, lo // D:hi // D, 0:D],
                        ps[:].rearrange("p (h d) -> p h d", d=D))
                else:
                    nc.vector.scalar_tensor_tensor(
                        va[:, lo // D:hi // D, 0:D],
                        ps[:].rearrange("p (h d) -> p h d", d=D), 1.0,
                        bv_b[:, lo:hi].rearrange("p (h d) -> p h d", d=D),
                        OP.mult, OP.add)
            nc.vector.memset(va[:, :, D:D + 1], 1.0)
            v_aug.append(va)

        # ---- attention per head ----
        ctx_bf = [work.tile([128, S], BF16, name=f"ctx{L}_{c}", tag=f"cx{c}",
                            bufs=1) for c in range(FC)]
        for hh in range(NH):
            mh, poff = hh // 2, (hh % 2) * D
            probs = []
            for u in range(ST):
                ps_sc = psum.tile([128, S], F32, name=f"sc{L}_{hh}_{u}",
                                  tag="acc", bufs=6)
                nc.tensor.matmul(ps_sc[:],
                                 k_bf[mh][poff:poff + D, ts(u, 128)],
                                 q_bf[mh][poff:poff + D, :],
                                 start=True, stop=True)
                pb = work.tile([128, S], BF16, name=f"pr{L}_{hh}_{u}",
                               tag="probs", bufs=4)
                nc.scalar.activation(out=pb[:], in_=ps_sc[:], func=AF.Exp,
                                     scale=1.0 / (D ** 0.5))
                probs.append(pb)
            ps_cx = psum.tile([D + 1, S], F32, name=f"cx{L}_{hh}",
                              tag="acc", bufs=6)
            for u in range(ST):
                nc.tensor.matmul(ps_cx[:], v_aug[u][:, hh, :], probs[u][:],
                                 start=(u == 0), stop=(u == ST - 1))
            rden_t = work.tile([1, S], F32, name=f"rden{L}_{hh}", tag="rden",
                               bufs=2)
            nc.vector.reciprocal(rden_t[:], ps_cx[D:D + 1, :])
            rdb = work.tile([128, S], F32, name=f"rdb{L}_{hh}", tag="rdb",
                            bufs=1)
            nc.gpsimd.partition_broadcast(rdb[:], rden_t[:])
            nc.vector.tensor_tensor(ctx_bf[mh][poff:poff + D, :],
                                    ps_cx[0:D, :], rdb[poff:poff + D, :],
                                    OP.mult)

        # ---- attention out + residual + LN1 ----
        x1 = h_f32_tiles(f"x1_{L}")
        for m in range(FC):
            ps = psum.tile([128, S], F32, name=f"pat{L}_{m}", tag="mm", bufs=2)
            for c in range(FC):
                nc.tensor.matmul(ps[:], wo[:, c, ts(m, 128)], ctx_bf[c][:],
                                 start=(c == 0), stop=(c == FC - 1))
            if simple:
                nc.vector.tensor_add(x1[m][:], ps[:], h_f32[m][:])
            else:
                nc.vector.scalar_tensor_tensor(x1[m][:], ps[:], bo[:, m:m + 1],
                                               h_f32[m][:], OP.add, OP.add)

        xa_f32, xa_bf = _layernorm_T(nc, psum, work, cfg, x1, g_attn, b_attn,
                                     h_f32_tiles, h_bf_tiles, f"ln1_{L}")

        # ---- FFN, up/down interleaved ----
        x2 = h_f32_tiles(f"x2_{L}")
        ps_d = [psum.tile([128, S], F32, name=f"psd{L}_{m}", tag="acc", bufs=6)
                for m in range(FC)]
        for jm in range(IC):
            ps_wi = psum.tile([128, S], F32, name=f"pwi{L}_{jm}", tag="mm",
                              bufs=2)
            for c in range(FC):
                nc.tensor.matmul(ps_wi[:], wi[:, c, ts(jm, 128)], xa_bf[c][:],
                                 start=(c == 0), stop=(c == FC - 1))
            gg = work.tile([128, S], BF16, name=f"gg{L}_{jm}", tag="gg", bufs=4)
            if simple:
                nc.scalar.activation(out=gg[:], in_=ps_wi[:], func=AF.Gelu)
            else:
                nc.scalar.activation(out=gg[:], in_=ps_wi[:], func=AF.Gelu,
                                     bias=bi[:, jm:jm + 1], scale=1.0)
            for m in range(FC):
                nc.tensor.matmul(ps_d[m][:], wd[:, jm, ts(m, 128)], gg[:],
                                 start=(jm == 0), stop=(jm == IC - 1))
        for m in range(FC):
            if simple:
                nc.vector.tensor_add(x2[m][:], ps_d[m][:], xa_f32[m][:])
            else:
                nc.vector.scalar_tensor_tensor(x2[m][:], ps_d[m][:],
                                               bd[:, m:m + 1], xa_f32[m][:],
                                               OP.add, OP.add)

        h_f32, h_bf = _layernorm_T(nc, psum, work, cfg, x2, g_out, b_out,
                                   h_f32_tiles, h_bf_tiles, f"ln2_{L}")

        for name, ap in dbg_outs.items():
            if name == f"hT{L}":
                for c in range(FC):
                    nc.sync.dma_start(ap[ts(c, 128), :], h_f32[c][:])
            if name == f"xaT{L}":
                for c in range(FC):
                    nc.sync.dma_start(ap[ts(c, 128), :], xa_f32[c][:])

    # ---------------- outputs ----------------
    ps_p = [psum.tile([128, 1], F32, name=f"psp{m}", tag="acc", bufs=6)
            for m in range(FC)]
    wp_chunks = t_["wp_d"].rearrange("(c p) j -> c p j", p=128)
    for c in range(FC):
        wpc = work.tile([128, H], BF16, name=f"wpc{c}", tag="wpc", bufs=1)
        nc.sync.dma_start(wpc[:], wp_chunks[c])
        for m in range(FC):
            nc.tensor.matmul(ps_p[m][:], wpc[:, ts(m, 128)], h_bf[c][:, 0:1],
                             start=(c == 0), stop=(c == FC - 1))
    for m in range(FC):
        pcol = work.tile([128, 1], F32, name=f"pcol{m}", tag="pcol", bufs=2)
        if simple:
            nc.vector.tensor_copy(pcol[:], ps_p[m][:])
        else:
            nc.vector.tensor_scalar(pcol[:], ps_p[m][:], bp[:, m:m + 1], None,
                                    OP.add)
        nc.sync.dma_start(
            t_["out_p"][0, ts(m, 128)].rearrange("(p q) -> p q", q=1), pcol[:])

    for t in range(ST):
        for c in range(FC):
            pt = psum.tile([128, 128], F32, name=f"pto{t}_{c}", tag="mm",
                           bufs=2)
            nc.tensor.transpose(out=pt[:], in_=h_f32[c][:, ts(t, 128)],
                                identity=ident[:])
            hnat = work.tile([128, 128], F32, name=f"hnat{t}_{c}",
                             tag="hnat", bufs=4)
            nc.vector.tensor_copy(hnat[:], pt[:])
            nc.sync.dma_start(t_["out_h"][ts(t, 128), ts(c, 128)], hnat[:])


def _layernorm_T(nc, psum, work, cfg, x, g_sb, b_sb, h_f32_tiles, h_bf_tiles,
                 gen):
    """Transposed-layout LayerNorm over the partition (feature) axis."""
    simple, ones_col = cfg["simple"], cfg["ones_col"]
    ps_sum = psum.tile([1, S], F32, name=f"pssum_{gen}", tag="acc", bufs=6)
    ps_sq = psum.tile([1, S], F32, name=f"pssq_{gen}", tag="acc", bufs=6)
    for c in range(FC):
        xb = work.tile([128, S], BF16, name=f"xb_{gen}_{c}", tag="xb", bufs=2)
        nc.scalar.copy(xb[:], x[c][:])
        sq = work.tile([128, S], BF16, name=f"sq_{gen}_{c}", tag="sq", bufs=2)
        nc.vector.tensor_tensor(sq[:], x[c][:], x[c][:], OP.mult)
        nc.tensor.matmul(ps_sum[:], ones_col[:], xb[:],
                         start=(c == 0), stop=(c == FC - 1))
        nc.tensor.matmul(ps_sq[:], ones_col[:], sq[:],
                         start=(c == 0), stop=(c == FC - 1))

    # packed [mean | rstd] row; one broadcast for both
    row = work.tile([1, 2 * S], F32, name=f"row_{gen}", tag="row", bufs=1)
    mean, rstd = row[:, 0:S], row[:, S:2 * S]
    nc.vector.tensor_scalar(mean, ps_sum[:], 1.0 / H, None, OP.mult)
    tr = work.tile([1, 2 * S], F32, name=f"tr_{gen}", tag="trow", bufs=1)
    t_row, y0 = tr[:, 0:S], tr[:, S:2 * S]
    # var = sumsq/H - mean^2 + eps
    nc.vector.scalar_tensor_tensor(t_row, mean, -1.0, mean, OP.mult, OP.mult)
    nc.vector.scalar_tensor_tensor(t_row, ps_sq[:], 1.0 / H, t_row,
                                   OP.mult, OP.add)
    cfg["rsqrt"](rstd, t_row, y0, 1)

    bc = work.tile([128, 2 * S], F32, name=f"bc_{gen}", tag="bc", bufs=1)
    nc.gpsimd.partition_broadcast(bc[:], row[:])
    mb, rb = bc[:, 0:S], bc[:, S:2 * S]

    out_f32 = h_f32_tiles(gen)
    out_bf = h_bf_tiles(gen)
    for c in range(FC):
        cen = work.tile([128, S], F32, name=f"cen_{gen}_{c}", tag="lnt", bufs=1)
        nc.vector.tensor_sub(cen[:], x[c][:], mb)
        if simple:
            nc.vector.tensor_tensor(out_f32[c][:], cen[:], rb, OP.mult)
        else:
            nrm = work.tile([128, S], F32, name=f"nrm_{gen}_{c}", tag="lnt2",
                            bufs=1)
            nc.vector.tensor_mul(nrm[:], cen[:], rb)
            nc.vector.tensor_scalar(out_f32[c][:], nrm[:], g_sb[:, c:c + 1],
                                    b_sb[:, c:c + 1], OP.mult, OP.add)
        nc.scalar.copy(out_bf[c][:], out_f32[c][:])
    return out_f32, out_bf


# ======================= host side =======================

_CACHE = {}


def _get_nc(n_layers=6, debug_outs=(), simple=True):
    key = (n_layers, tuple(sorted(debug_outs)), simple)
    if key not in _CACHE:
        _CACHE[key] = build_nc(n_layers, debug_outs, simple)
    return _CACHE[key]


class _Runner:
    """Jit-once runner with device-resident cached inputs (weights)."""

    def __init__(self, nc):
        import jax
        from jax.experimental.shard_map import shard_map
        from jax.sharding import Mesh, NamedSharding, PartitionSpec
        import concourse.mybir as _mybir
        from concourse.bass2jax import (_bass_exec_p, install_neuronx_cc_hook,
                                        partition_id_tensor)

        install_neuronx_cc_hook()
        self.jax = jax
        self.nc = nc
        in_names, out_names, out_avals = [], [], []
        partition_name = (nc.partition_id_tensor.name
                          if nc.partition_id_tensor else None)
        for alloc in nc.m.functions[0].allocations:
            if not isinstance(alloc, _mybir.MemoryLocationSet):
                continue
            name = alloc.memorylocations[0].name
            if alloc.kind == "ExternalInput":
                if name != partition_name:
                    in_names.append(name)
            elif alloc.kind == "ExternalOutput":
                shape = tuple(alloc.tensor_shape)
                dtype = _mybir.dt.np(alloc.dtype)
                out_names.append(name)
                out_avals.append(jax.core.ShapedArray(shape, dtype))
        self.in_names = list(in_names)
        self.out_names = out_names
        self.out_avals = out_avals
        n_params = len(in_names)
        n_outs = len(out_avals)
        all_names = in_names + out_names
        if partition_name is not None:
            all_names.append(partition_name)
        donate = tuple(range(n_params, n_params + n_outs))

        devices = jax.devices()[:N_CORES]
        assert len(devices) == N_CORES
        self.mesh = Mesh(np.asarray(devices), ("core",))
        self.sharding = NamedSharding(self.mesh, PartitionSpec("core"))

        def _body(*args):
            operands = list(args)
            if partition_name is not None:
                operands.append(partition_id_tensor())
            outs = _bass_exec_p.bind(
                *operands,
                out_avals=tuple(out_avals),
                in_names=tuple(all_names),
                out_names=tuple(out_names),
                lowering_input_output_aliases=(),
                sim_require_finite=True,
                sim_require_nnan=True,
                nc=nc,
            )
            return tuple(outs)

        in_specs = (PartitionSpec("core"),) * (n_params + n_outs)
        out_specs = (PartitionSpec("core"),) * n_outs
        self.sharded = jax.jit(
            shard_map(_body, mesh=self.mesh, in_specs=in_specs,
                      out_specs=out_specs, check_rep=False),
            donate_argnums=donate, keep_unused=True)
        self._dev_cache = {}

        import jax.numpy as jnp

        def _mkzeros():
            return tuple(
                jnp.zeros((N_CORES * a.shape[0], *a.shape[1:]), a.dtype)
                for a in out_avals)

        self._mkzeros = jax.jit(
            _mkzeros, out_shardings=tuple(self.sharding for _ in out_avals))

    def put(self, name, arrs_per_core):
        key = tuple(id(a) for a in arrs_per_core)
        hit = self._dev_cache.get(name)
        if hit is not None and hit[0] == key:
            return hit[1]
        cat = np.concatenate([np.asarray(a) for a in arrs_per_core], axis=0)
        dev = self.jax.device_put(cat, self.sharding)
        dev.block_until_ready()
        self._dev_cache[name] = (key, dev)
        return dev

    def zeros(self):
        z = list(self._mkzeros())
        self.jax.block_until_ready(z)
        return z

    def exec(self, dev_inputs, dev_zeros):
        outs = self.sharded(*dev_inputs, *dev_zeros)
        self.jax.block_until_ready(outs)
        return outs

    def __call__(self, in_maps):
        dev_inputs = [self.put(n, [m[n] for m in in_maps])
                      for n in self.in_names]
        outs = self.exec(dev_inputs, self.zeros())
        results = []
        host = [np.asarray(o) for o in outs]
        for c in range(N_CORES):
            results.append({
                name: host[i].reshape(N_CORES, *self.out_avals[i].shape)[c]
                for i, name in enumerate(self.out_names)})
        return results


def _get_runner(n_layers=6, debug_outs=(), simple=None, inputs=None):
    if simple is None:
        simple = _inputs_are_simple(inputs) if inputs is not None else True
    key = ("runner", n_layers, tuple(sorted(debug_outs)), simple)
    if key not in _CACHE:
        _CACHE[key] = _Runner(_get_nc(n_layers, debug_outs, simple))
    return _CACHE[key]


def _inputs_are_simple(inputs):
    try:
        zeros = all(not np.any(np.asarray(inputs[b]))
                    for b in ("bq", "bk", "bv", "bo", "bi", "bd", "bp",
                              "b_emb", "b_attn", "b_out"))
        ones = all(np.all(np.asarray(inputs[g]) == 1.0)
                   for g in ("g_emb", "g_attn", "g_out"))
        return bool(zeros and ones)
    except Exception:
        return False


_WEIGHT_SRC = ["W_word", "W_pos", "W_type", "Wq", "Wk", "Wv", "Wo", "Wi",
               "Wd", "Wp", "bq", "bk", "bv", "bo", "bi", "bd", "bp",
               "g_emb", "b_emb", "g_attn", "b_attn", "g_out", "b_out"]


def _prep_shared(inputs):
    key = tuple(id(inputs[n]) for n in _WEIGHT_SRC)
    hit = _CACHE.get("shared")
    if hit is not None and hit[0] == key:
        return hit[1]
    bf = ml_dtypes.bfloat16
    f32 = np.float32

    def a(name, dt=f32):
        return np.ascontiguousarray(np.asarray(inputs[name]).astype(dt))

    shared = {
        "W_word": a("W_word"), "W_pos": a("W_pos"), "W_type": a("W_type"),
        "Wq": a("Wq", bf), "Wk": a("Wk", bf), "Wv": a("Wv", bf),
        "Wo": a("Wo", bf), "Wi": a("Wi", bf), "Wd": a("Wd", bf),
        "Wp": a("Wp", bf),
        "bq": a("bq"), "bk": a("bk"), "bv": a("bv"), "bo": a("bo"),
        "bi": a("bi"), "bd": a("bd"), "bp": a("bp"),
        "g_emb": a("g_emb"), "b_emb": a("b_emb"),
        "g_attn": a("g_attn"), "b_attn": a("b_attn"),
        "g_out": a("g_out"), "b_out": a("b_out"),
    }
    _CACHE["shared"] = (key, shared)
    return shared


def _prep_in_maps(inputs):
    shared = _prep_shared(inputs)
    x = np.asarray(inputs["x"]).astype(np.int32)
    in_maps = []
    for c in range(N_CORES):
        m = dict(shared)
        m["x"] = np.ascontiguousarray(x[c:c + 1, :])
        in_maps.append(m)
    return in_maps


def run(inputs, n_layers=6, debug_outs=()):
    runner = _get_runner(n_layers, debug_outs, inputs=inputs)
    in_maps = _prep_in_maps(inputs)
    results = runner(in_maps)
    h = np.stack([results[c]["out_h"] for c in range(N_CORES)])
    pooled = np.stack([results[c]["out_pooled"][0] for c in range(N_CORES)])
    extras = {
        name: np.stack([results[c][name] for c in range(N_CORES)])
        for name in debug_outs
    }
    return (h, pooled), extras, results


def kernel(**inputs):
    (h, pooled), _, _ = run(inputs)
    return h.astype(np.float32), pooled.astype(np.float32)
